# revision 22
# baseline (speedup 1.0000x reference)
"""Custom GRU cell kernel for Trainium2, data-parallel over batch on 8 NeuronCores.

Layout strategy: everything on-device lives in [feature=128 partitions, batch free]
("transposed") layout so the six 128x128 weight matrices are the stationary matmul
operands and no on-device transposes are needed. The host pre-transposes x/h0 and
post-transposes the output history.

Per-step dataflow (per core, B_local=256, all tiles [128, 256] unless noted):
  PE : ps_rz[:,0:256]  = W_r.T x_t ; += U_r.T h     (one PSUM bank, [128,512])
       ps_rz[:,256:512]= W_z.T x_t ; += U_z.T h
       ps_xh = W_h.T x_t            (accumulation group left open)
       ps_mmh= U_h.T h
       ps_xh += I.T @ t1            (identity matmul folds the r*(U_h h) add)
  ACT: ru   = sigmoid(ps_rz)        ([128,512], r and u in one op)
       htil = tanh(ps_xh + b_h)
  DVE: t1 = ps_mmh * r ; g = u * dif ; e = g * a_bc ; h' = h + e
  GPS: dif = htil - h ; a_bc = partition_broadcast(a chunk) once per chunk
State h is bf16 and h' is written straight into the output chunk, DMA'd out in
[U, T_chunk, B_local] layout; matmul inputs (x, h, weights) are bf16, PSUM is f32.
"""

import sys

sys.path.insert(0, "/opt/trn_rl_repo")

import numpy as np
import ml_dtypes

import concourse.bass as bass  # noqa: F401  (import registers rust bindings)
import concourse.mybir as mybir
import concourse.tile as tile
from concourse import bacc
from concourse.bass_utils import run_bass_kernel_spmd

BF16 = mybir.dt.bfloat16
F16 = mybir.dt.float16
F32 = mybir.dt.float32
AF = mybir.ActivationFunctionType
OP = mybir.AluOpType

B, T, U = 2048, 200, 128
NCORES = 8
BL = B // NCORES  # 256 batch rows per core
TC = 25  # timesteps per chunk
NCHUNK = T // TC

# knobs (flipped during tuning)
USE_GPS_BCAST = False  # a-broadcast via gpsimd.partition_broadcast vs PE K=1 matmul
DIF_ON_GPS = False  # (htil - h) on GPSIMD vs VectorE

# set by test.py to collect profile info; kernel() stores results here
PROFILE = False
LAST_RESULT = None
LAST_IN_MAPS = None

_cache = {}


def _build(has_brz: bool, T_=T, TC_=TC, BL_=BL, reps=1):
    """Build + compile the per-core Bass program. has_brz: b_r/b_z nonzero path."""
    NCHUNK_ = T_ // TC_
    nc = bacc.Bacc("TRN2", target_bir_lowering=False)

    xt = nc.dram_tensor("xt", [U, T_, BL_], BF16, kind="ExternalInput")
    av = nc.dram_tensor("av", [U, T_ * BL_], BF16, kind="ExternalInput")
    h0t = nc.dram_tensor("h0t", [U, BL_], BF16, kind="ExternalInput")
    wcat = nc.dram_tensor("wcat", [9, U, U], BF16, kind="ExternalInput")
    ident_d = nc.dram_tensor("ident", [U, U], BF16, kind="ExternalInput")
    biases = nc.dram_tensor("biases", [U, 3], F32, kind="ExternalInput")
    outt = nc.dram_tensor("outt", [U, T_, BL_], BF16, kind="ExternalOutput")

    with tile.TileContext(nc) as tc:
        with (
            tc.tile_pool(name="const", bufs=1) as cpool,
            tc.tile_pool(name="xchunk", bufs=2) as xpool,
            tc.tile_pool(name="achunk", bufs=2) as apool,
            tc.tile_pool(name="abc", bufs=2) as abcpool,
            tc.tile_pool(name="ochunk", bufs=2) as opool,
            tc.tile_pool(name="work", bufs=4) as wpool,
            tc.tile_pool(name="psum", bufs=2, space="PSUM") as ppool,
        ):
            wts = []
            for i in range(9):
                wt = cpool.tile([U, U], BF16, tag=f"w{i}")
                nc.sync.dma_start(wt[:], wcat[i])
                wts.append(wt)
            w_r, u_r, w_z, u_z, w_h, u_h, nu_r, nu_z, nu_h = wts
            ident = cpool.tile([U, U], BF16, tag="ident")
            nc.sync.dma_start(ident[:], ident_d[:])
            btile = cpool.tile([U, 3], F32, tag="biases")
            nc.sync.dma_start(btile[:], biases[:])
            b_r_ap = btile[:, 0:1]
            b_z_ap = btile[:, 1:2]
            b_h_ap = btile[:, 2:3]
            h0tile = cpool.tile([U, BL_], BF16, tag="h0")
            nc.sync.dma_start(h0tile[:], h0t[:])

            for _rep in range(reps):
                xchs = {}

                def load_chunk(k):
                    if k >= NCHUNK_ or k in xchs:
                        return
                    t0, t1x = k * TC_, (k + 1) * TC_
                    xch = xpool.tile([U, TC_, BL_], BF16, tag="xch")
                    nc.sync.dma_start(xch[:], xt[:, t0:t1x, :])
                    ach = apool.tile([U, TC_ * BL_], BF16, tag="ach")
                    nc.sync.dma_start(ach[:], av[:, t0 * BL_ : t1x * BL_])
                    xchs[k] = (xch, ach)

                def emit_xside(t):
                    """x-dependent matmuls for step t (off the h critical path).
                    Also allocates the ps_mmh accumulator for step t."""
                    k, dt = divmod(t, TC_)
                    xch, ach = xchs[k]
                    xs = xch[:, dt, :]
                    # separate full-bank tiles for r and z: reads get
                    # tile-level deps, so sigmoid_r must not share a tile
                    # with the z-side writers (and two open accumulation
                    # groups must not share a 2KB PSUM zero region).
                    ps_r_full = ppool.tile([U, 2 * BL_], F32, tag="ps_r")
                    ps_r = ps_r_full[:, 0:BL_]
                    nc.tensor.matmul(ps_r, w_r[:], xs, start=True, stop=False)
                    ps_z_full = ppool.tile([U, 2 * BL_], F32, tag="ps_z")
                    ps_z = ps_z_full[:, 0:BL_]
                    nc.tensor.matmul(ps_z, w_z[:], xs, start=True, stop=False)
                    ps_xh_full = ppool.tile([U, 2 * BL_], F32, tag="ps_xh")
                    ps_xh = ps_xh_full[:, 0:BL_]
                    nc.tensor.matmul(ps_xh, w_h[:], xs, start=True, stop=False)
                    ps_mmh = ppool.tile([U, BL_], F32, tag="ps_mmh")
                    a_bc = ach[:, dt * BL_ : (dt + 1) * BL_]
                    return ps_r, ps_z, ps_xh, ps_mmh, a_bc

                h_prev = h0tile[:]
                load_chunk(0)
                pending = emit_xside(0)
                # bootstrap: h-side products for t=0 come straight from h0
                ps_r0, ps_z0, _, ps_mmh0, _ = pending
                nc.tensor.matmul(ps_r0, u_r[:], h_prev, start=False, stop=True)
                nc.tensor.matmul(ps_z0, u_z[:], h_prev, start=False, stop=True)
                nc.tensor.matmul(ps_mmh0[:], u_h[:], h_prev, start=True, stop=True)
                och = None
                for t in range(T_):
                    k, dt = divmod(t, TC_)
                    if dt == 0:
                        load_chunk(k + 1)
                        och = opool.tile([U, TC_, BL_], BF16, tag="och")
                    ps_r, ps_z, ps_xh, ps_mmh, a_bc = pending

                    # stage U_h h in SBUF early (input ready since the prev
                    # step's tail) so t1 runs in DVE 2x mode off the chain
                    mmh_sb = wpool.tile([U, BL_], F16, tag="mmh_sb")
                    nc.vector.tensor_copy(mmh_sb[:], ps_mmh[:])

                    r_sb = wpool.tile([U, BL_], BF16, tag="r_sb")
                    if has_brz:
                        nc.scalar.activation(r_sb[:], ps_r, AF.Sigmoid, bias=b_r_ap)
                    else:
                        nc.scalar.activation(r_sb[:], ps_r, AF.Sigmoid)
                    u_sb = wpool.tile([U, BL_], BF16, tag="u_sb")
                    if has_brz:
                        nc.scalar.activation(u_sb[:], ps_z, AF.Sigmoid, bias=b_z_ap)
                    else:
                        nc.scalar.activation(u_sb[:], ps_z, AF.Sigmoid)

                    t1 = wpool.tile([U, BL_], F16, tag="t1")
                    nc.vector.tensor_tensor(t1[:], r_sb[:], mmh_sb[:], OP.mult)
                    nc.tensor.matmul(ps_xh, ident[:], t1[:], start=False, stop=True)
                    if t + 1 < T_:
                        pending = emit_xside(t + 1)
                        ps_r_n, ps_z_n, _, ps_mmh_n, _ = pending

                    # shadow ops on DVE, ordered so none blocks m2:
                    # uhat = a*u, then m1 = (uhat-1)*h
                    uhat = wpool.tile([U, BL_], BF16, tag="uhat")
                    nc.vector.tensor_tensor(uhat[:], u_sb[:], a_bc, OP.mult)
                    m1 = wpool.tile([U, BL_], F16, tag="m1")
                    nc.vector.scalar_tensor_tensor(
                        m1[:], uhat[:], 1.0, h_prev, OP.subtract, OP.mult
                    )

                    htil = wpool.tile([U, BL_], BF16, tag="htil")
                    nc.scalar.activation(htil[:], ps_xh, AF.Tanh, bias=b_h_ap)
                    m2 = wpool.tile([U, BL_], F16, tag="m2")
                    nc.vector.tensor_tensor(m2[:], uhat[:], htil[:], OP.mult)
                    # linear tail: h' = m2 - m1 and U.h' = U.m2 + (-U).m1, so
                    # the next step's h-side products consume m1 (shadow) and
                    # m2 (hot) directly -- h' itself is off the critical path.
                    # r-gate matmuls first: sigmoid_r(t+1) only waits on those.
                    if t + 1 < T_:
                        nc.tensor.matmul(ps_r_n, nu_r[:], m1[:], start=False, stop=False)
                        nc.tensor.matmul(ps_r_n, u_r[:], m2[:], start=False, stop=True)
                        nc.tensor.matmul(ps_z_n, nu_z[:], m1[:], start=False, stop=False)
                        nc.tensor.matmul(ps_z_n, u_z[:], m2[:], start=False, stop=True)
                    hn = och[:, dt, :]
                    nc.vector.tensor_tensor(hn, m2[:], m1[:], OP.subtract)
                    if t + 1 < T_:
                        # U_h h' is consumed late (by t1(t+1)), so the plain
                        # product from h' makes the deadline and costs one mm
                        nc.tensor.matmul(ps_mmh_n[:], u_h[:], hn, start=True, stop=True)
                    h_prev = hn

                    if dt == TC_ - 1:
                        nc.sync.dma_start(outt[:, k * TC_ : (k + 1) * TC_, :], och[:])
                        xchs.pop(k, None)

    nc.compile()
    return nc


def kernel(inputs, h0, W_r, U_r, b_r, W_z, U_z, b_z, W_h, U_h, b_h):
    global LAST_RESULT
    inputs = np.asarray(inputs, dtype=np.float32)
    h0 = np.asarray(h0, dtype=np.float32)
    ws = [np.asarray(w, dtype=np.float32) for w in (W_r, U_r, W_z, U_z, W_h, U_h)]
    bs = [np.asarray(b, dtype=np.float32) for b in (b_r, b_z, b_h)]

    has_brz = bool(np.any(bs[0]) or np.any(bs[1]))
    key = has_brz
    if key not in _cache:
        _cache[key] = _build(has_brz)
    nc = _cache[key]

    bf = ml_dtypes.bfloat16
    # order: w_r, u_r, w_z, u_z, w_h, u_h, -u_r, -u_z, -u_h  (negated copies
    # let the linear tail accumulate U.h' = U.m2 + (-U).m1 in PSUM)
    wcat = np.stack(
        [w.astype(bf) for w in ws]
        + [(-ws[1]).astype(bf), (-ws[3]).astype(bf), (-ws[5]).astype(bf)]
    )  # [9, U, U]
    ident = np.eye(U, dtype=bf)
    biases = np.stack([bs[0], bs[1], bs[2]], axis=1).astype(np.float32)  # [U, 3]

    x = inputs[:, :, :U]  # [B, T, U]
    a = inputs[:, :, U]  # [B, T]

    in_maps = []
    for c in range(NCORES):
        sl = slice(c * BL, (c + 1) * BL)
        xt_c = np.ascontiguousarray(x[sl].transpose(2, 1, 0)).astype(bf)  # [U,T,BL]
        a_c = a[sl].T.astype(bf).reshape(1, T * BL)  # [1, T*BL]
        a_bc = np.ascontiguousarray(np.broadcast_to(a_c, (U, T * BL)))  # [U, T*BL]
        h0t_c = np.ascontiguousarray(h0[sl].T).astype(bf)  # [U, BL]
        in_maps.append(
            {
                "xt": xt_c,
                "av": a_bc,
                "h0t": h0t_c,
                "wcat": wcat,
                "ident": ident,
                "biases": biases,
            }
        )

    res = run_bass_kernel_spmd(nc, in_maps, list(range(NCORES)), trace=PROFILE)
    global LAST_IN_MAPS
    LAST_IN_MAPS = in_maps
    LAST_RESULT = res

    out = np.empty((B, T, U), dtype=np.float32)
    for c in range(NCORES):
        sl = slice(c * BL, (c + 1) * BL)
        # outt: [U, T, BL] bf16 -> [BL, T, U] f32
        out[sl] = res.results[c]["outt"].astype(np.float32).transpose(2, 1, 0)
    return out



# revision 23
# speedup vs baseline: 1.2249x; 1.2249x over previous
"""Custom GRU cell kernel for Trainium2, data-parallel over batch on 8 NeuronCores.

Layout: everything on-device lives in [feature=128 partitions, batch free]
("transposed") layout so the six 128x128 weight matrices are the stationary
matmul operands and no on-device transposes are needed. The host pre-transposes
x/h0 (and replicates the attention scores a across all 128 partitions) and
post-transposes the output history.

Per-step dataflow (per core, B_local=256; engines are strictly in-order, so
issue order per queue is the schedule):
  PE : ps_r += U_r.T h ; ps_z += U_z.T h ; ps_mmh = U_h.T h
       (ps_r/ps_z/ps_xh were seeded with W_*.T x_t one step ahead;
        separate full-bank PSUM tiles per gate so sigmoid_r only waits U_r)
       ps_xh += I.T @ t1            (identity matmul folds the r*(U_h h) add)
  DVE: mmh_sb = copy(ps_mmh)        (early, off-chain -> t1 gets 2x mode)
       t1 = r * mmh_sb ; uhat = u * a_bc ; m1 = (uhat-1)*h
       m2 = uhat*htil ; h' = m2 - m1
  ACT: r = sigmoid(ps_r) ; u = sigmoid(ps_z) ; htil = tanh(ps_xh + b_h)
State h is bf16, written straight into the output chunk ([U, T_chunk, B_local]
layout); matmul inputs are bf16, PSUM is f32.
"""

import sys

sys.path.insert(0, "/opt/trn_rl_repo")

import numpy as np
import ml_dtypes

import concourse.bass as bass  # noqa: F401  (import registers rust bindings)
import concourse.mybir as mybir
import concourse.tile as tile
from concourse import bacc
from concourse.bass_utils import run_bass_kernel_spmd

BF16 = mybir.dt.bfloat16
F16 = mybir.dt.float16
F32 = mybir.dt.float32
AF = mybir.ActivationFunctionType
OP = mybir.AluOpType

B, T, U = 2048, 200, 128
NCORES = 8
BL = B // NCORES  # 256 batch rows per core
TC = 25  # timesteps per chunk
NCHUNK = T // TC

# set by test.py to collect profile info; kernel() stores results here
PROFILE = False
LAST_RESULT = None
LAST_IN_MAPS = None

_cache = {}


def _build(has_brz: bool, T_=T, TC_=TC, BL_=BL, reps=1):
    """Build + compile the per-core Bass program. has_brz: b_r/b_z nonzero path."""
    NCHUNK_ = T_ // TC_
    nc = bacc.Bacc("TRN2", target_bir_lowering=False)

    xt = nc.dram_tensor("xt", [U, T_, BL_], BF16, kind="ExternalInput")
    av = nc.dram_tensor("av", [U, T_ * BL_], BF16, kind="ExternalInput")
    h0t = nc.dram_tensor("h0t", [U, BL_], BF16, kind="ExternalInput")
    wcat = nc.dram_tensor("wcat", [6, U, U], BF16, kind="ExternalInput")
    ident_d = nc.dram_tensor("ident", [U, U], BF16, kind="ExternalInput")
    biases = nc.dram_tensor("biases", [U, 3], F32, kind="ExternalInput")
    outt = nc.dram_tensor("outt", [U, T_, BL_], BF16, kind="ExternalOutput")

    with tile.TileContext(nc) as tc:
        with (
            tc.tile_pool(name="const", bufs=1) as cpool,
            tc.tile_pool(name="xchunk", bufs=2) as xpool,
            tc.tile_pool(name="achunk", bufs=2) as apool,
            tc.tile_pool(name="ochunk", bufs=2) as opool,
            tc.tile_pool(name="work", bufs=4) as wpool,
            tc.tile_pool(name="psum", bufs=2, space="PSUM") as ppool,
        ):
            wts = []
            for i in range(6):
                wt = cpool.tile([U, U], BF16, tag=f"w{i}")
                nc.sync.dma_start(wt[:], wcat[i])
                wts.append(wt)
            w_r, u_r, w_z, u_z, w_h, u_h = wts
            ident = cpool.tile([U, U], BF16, tag="ident")
            nc.sync.dma_start(ident[:], ident_d[:])
            btile = cpool.tile([U, 3], F32, tag="biases")
            nc.sync.dma_start(btile[:], biases[:])
            b_r_ap = btile[:, 0:1]
            b_z_ap = btile[:, 1:2]
            b_h_ap = btile[:, 2:3]
            h0tile = cpool.tile([U, BL_], BF16, tag="h0")
            nc.sync.dma_start(h0tile[:], h0t[:])

            for _rep in range(reps):
                xchs = {}

                def load_chunk(k):
                    if k >= NCHUNK_ or k in xchs:
                        return
                    t0, t1x = k * TC_, (k + 1) * TC_
                    xch = xpool.tile([U, TC_, BL_], BF16, tag="xch")
                    nc.sync.dma_start(xch[:], xt[:, t0:t1x, :])
                    ach = apool.tile([U, TC_ * BL_], BF16, tag="ach")
                    nc.sync.dma_start(ach[:], av[:, t0 * BL_ : t1x * BL_])
                    xchs[k] = (xch, ach)

                def emit_xside(t):
                    """x-dependent matmuls for step t (off the h critical path).
                    Separate full-bank PSUM tiles per gate: tile-level read
                    deps mean sigmoid_r then only waits on the U_r matmul, and
                    concurrently-open accumulation groups never share a 2KB
                    PSUM zero region."""
                    k, dt = divmod(t, TC_)
                    xch, ach = xchs[k]
                    xs = xch[:, dt, :]
                    ps_r_full = ppool.tile([U, 2 * BL_], F32, tag="ps_r")
                    ps_r = ps_r_full[:, 0:BL_]
                    nc.tensor.matmul(ps_r, w_r[:], xs, start=True, stop=False)
                    ps_z_full = ppool.tile([U, 2 * BL_], F32, tag="ps_z")
                    ps_z = ps_z_full[:, 0:BL_]
                    nc.tensor.matmul(ps_z, w_z[:], xs, start=True, stop=False)
                    ps_xh_full = ppool.tile([U, 2 * BL_], F32, tag="ps_xh")
                    ps_xh = ps_xh_full[:, 0:BL_]
                    nc.tensor.matmul(ps_xh, w_h[:], xs, start=True, stop=False)
                    a_bc = ach[:, dt * BL_ : (dt + 1) * BL_]
                    return ps_r, ps_z, ps_xh, a_bc

                h_prev = h0tile[:]
                load_chunk(0)
                pending = emit_xside(0)
                och = None
                for t in range(T_):
                    k, dt = divmod(t, TC_)
                    if dt == 0:
                        load_chunk(k + 1)
                        och = opool.tile([U, TC_, BL_], BF16, tag="och")
                    ps_r, ps_z, ps_xh, a_bc = pending

                    # h-side products; r first so sigmoid_r unblocks earliest
                    ps_mmh = ppool.tile([U, BL_], F32, tag="ps_mmh")
                    nc.tensor.matmul(ps_r, u_r[:], h_prev, start=False, stop=True)
                    nc.tensor.matmul(ps_z, u_z[:], h_prev, start=False, stop=True)
                    nc.tensor.matmul(ps_mmh[:], u_h[:], h_prev, start=True, stop=True)

                    r_sb = wpool.tile([U, BL_], BF16, tag="r_sb")
                    if has_brz:
                        nc.scalar.activation(r_sb[:], ps_r, AF.Sigmoid, bias=b_r_ap)
                    else:
                        nc.scalar.activation(r_sb[:], ps_r, AF.Sigmoid)
                    u_sb = wpool.tile([U, BL_], BF16, tag="u_sb")
                    if has_brz:
                        nc.scalar.activation(u_sb[:], ps_z, AF.Sigmoid, bias=b_z_ap)
                    else:
                        nc.scalar.activation(u_sb[:], ps_z, AF.Sigmoid)

                    # DVE queue: copy (inputs ready at step start), then the
                    # chain ops in dependency order so nothing queue-blocks.
                    mmh_sb = wpool.tile([U, BL_], BF16, tag="mmh_sb")
                    nc.vector.tensor_copy(mmh_sb[:], ps_mmh[:])
                    t1 = wpool.tile([U, BL_], BF16, tag="t1")
                    nc.vector.tensor_tensor(t1[:], r_sb[:], mmh_sb[:], OP.mult)
                    nc.tensor.matmul(ps_xh, ident[:], t1[:], start=False, stop=True)
                    if t + 1 < T_:
                        pending = emit_xside(t + 1)

                    uhat = wpool.tile([U, BL_], BF16, tag="uhat")
                    nc.vector.tensor_tensor(uhat[:], u_sb[:], a_bc, OP.mult)
                    m1 = wpool.tile([U, BL_], BF16, tag="m1")
                    nc.vector.scalar_tensor_tensor(
                        m1[:], uhat[:], 1.0, h_prev, OP.subtract, OP.mult
                    )
                    htil = wpool.tile([U, BL_], BF16, tag="htil")
                    nc.scalar.activation(htil[:], ps_xh, AF.Tanh, bias=b_h_ap)
                    # on-chain tail: hn = uhat*htil - (uhat-1)*h
                    m2 = wpool.tile([U, BL_], BF16, tag="m2")
                    nc.vector.tensor_tensor(m2[:], uhat[:], htil[:], OP.mult)
                    hn = och[:, dt, :]
                    nc.vector.tensor_tensor(hn, m2[:], m1[:], OP.subtract)
                    h_prev = hn

                    if dt == TC_ - 1:
                        nc.sync.dma_start(outt[:, k * TC_ : (k + 1) * TC_, :], och[:])
                        xchs.pop(k, None)

    nc.compile()
    return nc


def kernel(inputs, h0, W_r, U_r, b_r, W_z, U_z, b_z, W_h, U_h, b_h):
    global LAST_RESULT
    inputs = np.asarray(inputs, dtype=np.float32)
    h0 = np.asarray(h0, dtype=np.float32)
    ws = [np.asarray(w, dtype=np.float32) for w in (W_r, U_r, W_z, U_z, W_h, U_h)]
    bs = [np.asarray(b, dtype=np.float32) for b in (b_r, b_z, b_h)]

    has_brz = bool(np.any(bs[0]) or np.any(bs[1]))
    key = has_brz
    if key not in _cache:
        _cache[key] = _build(has_brz)
    nc = _cache[key]

    bf = ml_dtypes.bfloat16
    wcat = np.stack([w.astype(bf) for w in ws])  # [6, U, U]
    ident = np.eye(U, dtype=bf)
    biases = np.stack([bs[0], bs[1], bs[2]], axis=1).astype(np.float32)  # [U, 3]

    x = inputs[:, :, :U]  # [B, T, U]
    a = inputs[:, :, U]  # [B, T]

    in_maps = []
    for c in range(NCORES):
        sl = slice(c * BL, (c + 1) * BL)
        xt_c = np.ascontiguousarray(x[sl].transpose(2, 1, 0)).astype(bf)  # [U,T,BL]
        a_c = a[sl].T.astype(bf).reshape(1, T * BL)  # [1, T*BL]
        a_bc = np.ascontiguousarray(np.broadcast_to(a_c, (U, T * BL)))  # [U, T*BL]
        h0t_c = np.ascontiguousarray(h0[sl].T).astype(bf)  # [U, BL]
        in_maps.append(
            {
                "xt": xt_c,
                "av": a_bc,
                "h0t": h0t_c,
                "wcat": wcat,
                "ident": ident,
                "biases": biases,
            }
        )

    res = run_bass_kernel_spmd(nc, in_maps, list(range(NCORES)), trace=PROFILE)
    global LAST_IN_MAPS
    LAST_IN_MAPS = in_maps
    LAST_RESULT = res

    out = np.empty((B, T, U), dtype=np.float32)
    for c in range(NCORES):
        sl = slice(c * BL, (c + 1) * BL)
        # outt: [U, T, BL] bf16 -> [BL, T, U] f32
        out[sl] = res.results[c]["outt"].astype(np.float32).transpose(2, 1, 0)
    return out
